# revision 39
# baseline (speedup 1.0000x reference)
"""Trainium2 Bass kernel for Exaone4-style GQA attention block (T=2048, HID=4096,
H=32 q-heads, HK=8 kv-heads, D=128, sliding window 1023, QK-RMSNorm + NeoX RoPE).

Sharding: tensor-parallel over heads across 8 NeuronCores. Core m owns q-heads
[4m, 4m+4) and kv-head m (GQA group-aligned), plus the matching o_proj column
slice; per-core partial outputs are summed on the host (the all-reduce).

Device layout notes:
 - qkv projection is computed transposed ([feature, t]) so attention works in
   the S^T = K^T.T @ Q^T layout; softmax sums over the partition axis are done
   with ones-vector matmuls on the PE, and PV consumes exp(S^T) directly.
 - RMSNorm scale and RoPE are fused via host-precomputed [128, T] cos/sin
   tables (norm weights + 1/sqrt(D) folded in); the RoPE half rotation runs on
   the raw projection (rotation commutes with the per-column norm scale), so
   the SBUF->SBUF rotate DMAs issue as soon as qkv PSUM results are copied out.
 - 1/sqrt(ms+eps) is computed as exp(-0.5*ln(ms+eps)) so every ACT-engine op
   lives in the natural_log_exp table set (no ACT table reloads vs softmax Exp).
 - Schedule keeps the PE dense: per iteration the PE stream is
   [o_proj(tb-1)] [attention(tb)] [v-transpose+rms(tb+1)] [qkv(tb+2)], with
   weight/x DMAs chunk-interleaved so tb=0 compute starts ~4us in.
 - All large matmuls use bf16 operands with fp32 PSUM accumulation.
"""

import sys

import numpy as np

if "/opt/trn_rl_repo" not in sys.path:
    sys.path.insert(0, "/opt/trn_rl_repo")

import ml_dtypes

BF16 = ml_dtypes.bfloat16
# fp8 was evaluated and rejected: the correctness gate is max-abs error and
# the largest outputs (peaked-softmax rows where attn == v) inherit fp8's ~4%
# relative error directly (measured rel 0.048 vs the 0.02 gate). bf16 only.

HID = 4096
H = 32
HK = 8
D = 128
WIN = 1023
THETA = 1000000.0
EPS = 1e-6
SCALE = D ** -0.5
M = 8            # cores
QH = H // M      # q heads per core (4)
NJ = QH + 2      # j-blocks in qkv^T output (4 q + 1 k + 1 v)
TB = 512         # t free-dim block
NEG = -1.0e30

_PROG_CACHE = {}


def _build_program(T):
    """Build the (single-core SPMD) Bass program for sequence length T."""
    from contextlib import ExitStack

    import concourse.bass as bass  # noqa: F401
    import concourse.tile as tile
    from concourse import bacc, mybir

    f32 = mybir.dt.float32
    bf = mybir.dt.bfloat16

    NT = T // TB          # number of t blocks
    NC = HID // 128       # contraction chunks
    NOB = HID // 128      # output row blocks

    nc = bacc.Bacc(
        "TRN2",
        target_bir_lowering=False,
        debug=False,
        enable_asserts=False,
        num_devices=M,
    )

    # x pre-tiled on host: block (tb, cq) = [128, 4*TB], 4 c-chunks interleaved
    # per partition row (contiguous per partition per DMA)
    xT_h = nc.dram_tensor(
        "xT", [(T // TB) * (HID // 512) * 128, 4 * TB], bf, kind="ExternalInput"
    )
    # qkv weights pre-tiled the same way: chunk k = [128, 4*NJ*128] contiguous
    wq_h = nc.dram_tensor(
        "wqkvT", [(NC // 4) * 128, 4 * NJ * 128], bf, kind="ExternalInput"
    )
    wo_h = nc.dram_tensor("woT", [QH * 128, HID], bf, kind="ExternalInput")
    cwq_h = nc.dram_tensor("cwq", [128, T], bf, kind="ExternalInput")
    swq_h = nc.dram_tensor("swq", [128, T], bf, kind="ExternalInput")
    cwk_h = nc.dram_tensor("cwk", [128, T], bf, kind="ExternalInput")
    swk_h = nc.dram_tensor("swk", [128, T], bf, kind="ExternalInput")
    maskd_h = nc.dram_tensor("maskd", [128, 128], f32, kind="ExternalInput")
    maskw_h = nc.dram_tensor("maskw", [128, 128], f32, kind="ExternalInput")
    # out pre-tiled: block (tb, obp) = [128, 2*TB] (ob pairs interleaved per row)
    outT_h = nc.dram_tensor(
        "outT", [(T // TB) * (HID // 256) * 128, 2 * TB], bf, kind="ExternalOutput"
    )

    xTr = xT_h.ap().rearrange("(b p) u -> b p u", p=128)
    wqr = wq_h.ap().rearrange("(k p) u -> k p u", p=128)
    wor = wo_h.ap().rearrange("(jc p) o -> p jc o", p=128)
    outr = outT_h.ap().rearrange("(b p) u -> b p u", p=128)

    mult = mybir.AluOpType.mult
    add = mybir.AluOpType.add
    Exp = mybir.ActivationFunctionType.Exp
    Ln = mybir.ActivationFunctionType.Ln
    Square = mybir.ActivationFunctionType.Square

    with tile.TileContext(nc) as tc, ExitStack() as ctx:
        singles = ctx.enter_context(tc.tile_pool(name="singles", bufs=1))
        persist = ctx.enter_context(tc.tile_pool(name="persist", bufs=1))
        xpool = ctx.enter_context(tc.tile_pool(name="xpool", bufs=3))
        stpool = ctx.enter_context(tc.tile_pool(name="stpool", bufs=1))
        ropep = ctx.enter_context(tc.tile_pool(name="ropep", bufs=1))
        espool = ctx.enter_context(tc.tile_pool(name="espool", bufs=5))
        outp = ctx.enter_context(tc.tile_pool(name="outp", bufs=2))
        smallp = ctx.enter_context(tc.tile_pool(name="smallp", bufs=2))
        # PSUM: every tile is <= one bank; a single tag with 8 rotating slots
        # covers all 8 banks and lets phases overlap freely.
        psum = ctx.enter_context(tc.tile_pool(name="psum", bufs=8, space="PSUM"))
        drp = ctx.enter_context(tc.tile_pool(name="drp", bufs=2, space="DRAM"))

        # ---- tiny resident constants (sync queue, land first) -------------
        maskd_sb = singles.tile([128, 128], f32)
        nc.sync.dma_start(maskd_sb, maskd_h.ap())
        maskw_sb = singles.tile([128, 128], f32)
        nc.sync.dma_start(maskw_sb, maskw_h.ap())
        ones_bf = singles.tile([128, 1], bf)
        nc.vector.memset(ones_bf, 1.0)
        eps_sb = singles.tile([128, 1], f32)
        nc.vector.memset(eps_sb, EPS)

        # ---- rope tables on the gpsimd queue (parallel with w) ------------
        cwq_sb = singles.tile([128, T], bf)
        nc.gpsimd.dma_start(cwq_sb, cwq_h.ap())
        swq_sb = singles.tile([128, T], bf)
        nc.gpsimd.dma_start(swq_sb, swq_h.ap())
        cwk_sb = singles.tile([128, T], bf)
        nc.gpsimd.dma_start(cwk_sb, cwk_h.ap())
        swk_sb = singles.tile([128, T], bf)
        nc.gpsimd.dma_start(swk_sb, swk_h.ap())

        # ---- o_proj weights (DMA deferred until after phase_b(0) so the
        # startup bandwidth goes to qkv weights + x) -------------------------
        wo_sb = singles.tile([128, QH, HID], bf)

        # ---- qkv weights: separate chunk tiles so the first matmuls only
        # wait on the first chunk DMA, not the whole load --------------------
        w_chunks = [
            singles.tile([128, 4, NJ * 128], bf, name=f"w_chunk{k}")
            for k in range(NC // 4)
        ]
        w_loaded = [False] * (NC // 4)

        # ---- persistent activations ---------------------------------------
        qT = persist.tile([128, QH, T], bf)     # rope'd+normed q^T
        kT = persist.tile([128, T], bf)         # rope'd+normed k^T
        Vt = persist.tile([128, T // 128, 128], bf)  # v in [s, d] layout

        stages = {}

        def qkv_mm(tb, wide=False):
            """qkv matmuls for t block tb; also emits the psum->stage copies,
            the rope half-rotation DMAs, and the squared-stage muls.

            wide=True runs all 6 j-blocks in one pass over x (6 PSUM banks,
            half the x DMA) - used for tb=0 where nothing else needs PSUM."""
            t0 = tb * TB
            stage = stpool.tile([128, NJ, TB], bf, tag="stage", name=f"stage_{tb}")
            rot = ropep.tile([128, QH + 1, TB], bf, tag="rot", name=f"rot_{tb}")
            sq = stpool.tile([128, QH + 1, TB], bf, tag="sq", name=f"sq_{tb}")
            stages[tb] = (stage, rot, sq)

            groups = [tuple(range(NJ))] if wide else [(0, 1, 2), (3, 4, 5)]
            for js in groups:
                ps_g = [
                    psum.tile([128, TB], f32, name=f"psqkv_{tb}_{j}", tag="bank")
                    for j in js
                ]
                for cq in range(NC // 4):
                    if not w_loaded[cq]:
                        nc.sync.dma_start(
                            w_chunks[cq],
                            wqr[cq].rearrange("p (ci u) -> p ci u", u=NJ * 128),
                        )
                        w_loaded[cq] = True
                    xc = xpool.tile(
                        [128, 4, TB], bf, tag="xc", name=f"xc_{tb}_{js[0]}_{cq}"
                    )
                    nc.sync.dma_start(
                        xc,
                        xTr[tb * (NC // 4) + cq].rearrange("p (ci u) -> p ci u", u=TB),
                    )
                    for ci in range(4):
                        c = 4 * cq + ci
                        for ji, j in enumerate(js):
                            nc.tensor.matmul(
                                ps_g[ji],
                                lhsT=w_chunks[cq][:, ci, j * 128 : (j + 1) * 128],
                                rhs=xc[:, ci, :],
                                start=(c == 0),
                                stop=(c == NC - 1),
                            )
                for ji, j in enumerate(js):
                    if j <= QH:
                        # sq straight from PSUM (ACT Square; DVE cannot read
                        # two PSUM operands) so the rms rowsum does not wait
                        # on the stage copy
                        nc.scalar.activation(sq[:, j], ps_g[ji], Square)
                    # stage copy on ACT (Copy shares the Exp table set) keeps
                    # DVE free for masks/rope
                    nc.scalar.copy(stage[:, j], ps_g[ji])
                    if j <= QH:
                        # rope rotation on the raw projection (scale commutes)
                        nc.gpsimd.dma_start(rot[0:64, j, :], stage[64:128, j, :])
                        nc.gpsimd.dma_start(rot[64:128, j, :], stage[0:64, j, :])
                    else:
                        # v: [d, t] -> [s, d] via DMA transpose (sync queue)
                        for u in range(TB // 128):
                            nc.sync.dma_start(
                                Vt[:, tb * (TB // 128) + u, :],
                                stage[:, j, u * 128 : (u + 1) * 128],
                                transpose=True,
                            )

        def tail(tb):
            """rms scale (PE rowsum + ACT ln/exp + bcast) and rope combine
            for t block tb. PE part is tiny (5 ones-matmuls)."""
            t0 = tb * TB
            ts_ = slice(t0, t0 + TB)
            stage, rot, sq = stages.pop(tb)

            scl = smallp.tile(
                [1, (QH + 1) * TB], bf, tag="scl", bufs=1, name=f"scl_{tb}"
            )
            # batch by function so the ACT table loads once per function group
            # (the table-load pass reloads per canonical set on each switch)
            ps_sss = []
            for j in range(QH + 1):
                ps_ss = psum.tile([1, TB], f32, name=f"psss_{tb}_{j}", tag="bank")
                nc.tensor.matmul(ps_ss, lhsT=ones_bf, rhs=sq[:, j], start=True, stop=True)
                ps_sss.append(ps_ss)
            for j in range(QH + 1):
                # 1/sqrt(x) = exp(-0.5*ln(x)); Ln runs in place in PSUM
                nc.scalar.activation(
                    ps_sss[j], ps_sss[j], Ln, bias=eps_sb[0:1, :], scale=1.0 / D
                )
            for j in range(QH + 1):
                nc.scalar.activation(
                    scl[:, j * TB : (j + 1) * TB], ps_sss[j], Exp, scale=-0.5
                )
            scl_dr = drp.tile([1, (QH + 1) * TB], bf, tag="scl_dr", name=f"scldr_{tb}")
            nc.gpsimd.dma_start(scl_dr, scl)
            sclb = ropep.tile(
                [128, (QH + 1) * TB], bf, tag="sclb", name=f"sclb_{tb}"
            )
            nc.gpsimd.dma_start(sclb, scl_dr.to_broadcast([128, (QH + 1) * TB]))

            # rope combine first (no sclb dependency), scale muls last
            r1s = []
            for j in range(QH + 1):
                cw = cwq_sb if j < QH else cwk_sb
                sw = swq_sb if j < QH else swk_sb
                r1 = ropep.tile([128, TB], f32, tag=f"r1_{j}", name=f"r1_{tb}_{j}")
                nc.vector.tensor_tensor(r1, stage[:, j], cw[:, ts_], mult)
                r2 = ropep.tile([128, TB], f32, tag="r2", bufs=2, name=f"r2_{tb}_{j}")
                nc.vector.tensor_tensor(r2, rot[:, j], sw[:, ts_], mult)
                nc.vector.tensor_tensor(r1, r1, r2, add)
                r1s.append(r1)
            for j in range(QH + 1):
                dest = qT[:, j, ts_] if j < QH else kT[:, ts_]
                nc.vector.tensor_tensor(
                    dest, r1s[j], sclb[:, j * TB : (j + 1) * TB], mult
                )

        attnTs = {}

        def phase_b(tb, u0=0, u1=TB):
            """attention for t block tb, query columns [u0, u1) (attnT kept
            for phase_c).

            Head pairs; the S matmul for block o+1 is emitted before the
            PV/rowsum matmuls for block o so the mask-add->exp round trip
            hides behind PE work. o = sb - 4*tb; o=0 (full col range) FIRST
            so start=True PV/rowsum matmuls cover the whole written range."""
            t0 = tb * TB
            W = u1 - u0
            o_min = max(-8, -((1023 - u0 + 127) // 128))
            o_max = (u1 - 1) // 128
            obs = [0] + [
                o for o in range(o_min, o_max + 1) if o != 0 and 4 * tb + o >= 0
            ]
            nob = len(obs)
            if u0 == 0:
                attnT = outp.tile(
                    [128, QH, TB], bf, tag="attnT", name=f"attnT_{tb}"
                )
                attnTs[tb] = attnT
            else:
                attnT = attnTs[tb]
            rs_dr = drp.tile([1, QH * W], bf, tag="rs_dr", name=f"rsdr_{tb}_{u0}")

            def col_range(o):
                if o >= 0:
                    b0, b1 = 128 * o, TB
                elif o >= -4:
                    b0, b1 = 0, TB
                else:
                    b0, b1 = 0, 128 * (o + 9)
                return max(b0, u0), min(b1, u1)

            for hp in range(QH // 2):
                heads = (2 * hp, 2 * hp + 1)
                pvs = {
                    h: psum.tile([128, TB], f32, name=f"pspv_{tb}_{h}", tag="bank")
                    for h in heads
                }
                rss = {
                    h: psum.tile([1, TB], f32, name=f"psr_{tb}_{h}", tag="bank")
                    for h in heads
                }
                es_tiles = {}

                def emit_S(idx):
                    o = obs[idx]
                    sb = 4 * tb + o
                    c0, c1 = col_range(o)
                    for h in heads:
                        ps_s = psum.tile(
                            [128, TB], f32, name=f"pss_{tb}_{h}_{idx}", tag="bank"
                        )
                        nc.tensor.matmul(
                            ps_s[:, c0:c1],
                            lhsT=kT[:, sb * 128 : (sb + 1) * 128],
                            rhs=qT[:, h, t0 + c0 : t0 + c1],
                            start=True,
                            stop=True,
                        )
                        if o >= 0 and u0 <= 128 * o < u1:
                            # causal strip at cols [128o, 128o+128)
                            us = 128 * o
                            nc.vector.tensor_tensor(
                                ps_s[:, us : us + 128], ps_s[:, us : us + 128],
                                maskd_sb, add,
                            )
                        elif o <= -5 and u0 <= 128 * (o + 8) < u1:
                            # window strip
                            us = 128 * (o + 8)
                            nc.vector.tensor_tensor(
                                ps_s[:, us : us + 128], ps_s[:, us : us + 128],
                                maskw_sb, add,
                            )
                        es = espool.tile(
                            [128, TB], bf, tag="es", name=f"es_{tb}_{h}_{idx}"
                        )
                        nc.scalar.activation(es[:, c0:c1], ps_s[:, c0:c1], Exp)
                        es_tiles[(idx, h)] = es

                emit_S(0)
                if nob > 1:
                    emit_S(1)
                for idx, o in enumerate(obs):
                    if idx + 2 < nob:
                        emit_S(idx + 2)
                    sb = 4 * tb + o
                    c0, c1 = col_range(o)
                    first = idx == 0
                    last = idx == nob - 1
                    for h in heads:
                        es = es_tiles.pop((idx, h))
                        nc.tensor.matmul(
                            pvs[h][:, c0:c1],
                            lhsT=Vt[:, sb, :],
                            rhs=es[:, c0:c1],
                            start=first,
                            stop=last,
                            skip_group_check=True,
                        )
                        nc.tensor.matmul(
                            rss[h][:, c0:c1],
                            lhsT=ones_bf,
                            rhs=es[:, c0:c1],
                            start=first,
                            stop=last,
                            skip_group_check=True,
                        )

                for hi, h in enumerate(heads):
                    # park pre-normalized PV in attnT (normalized in place at
                    # phase end once 1/rowsum is broadcast)
                    nc.vector.tensor_copy(attnT[:, h, u0:u1], pvs[h][:, u0:u1])
                    rtmp = smallp.tile([1, TB], f32, tag="rtmp", bufs=2,
                                       name=f"rtmp_{tb}_{h}_{u0}")
                    nc.vector.reciprocal_approx_fast(rtmp[:, :W], rss[h][:, u0:u1])
                    rtmpb = smallp.tile([1, TB], bf, tag="rtmpb", bufs=2,
                                        name=f"rtmpb_{tb}_{h}_{u0}")
                    nc.vector.tensor_copy(rtmpb[:, :W], rtmp[:, :W])
                    nc.gpsimd.dma_start(rs_dr[:, h * W : (h + 1) * W], rtmpb[:, :W])

            rb4 = ropep.tile([128, QH * TB], bf, tag="rb4", name=f"rb4_{tb}_{u0}")
            nc.gpsimd.dma_start(rb4[:, : QH * W], rs_dr.to_broadcast([128, QH * W]))
            for h in range(QH):
                nc.vector.tensor_tensor(attnT[:, h, u0:u1], attnT[:, h, u0:u1],
                                        rb4[:, h * W : (h + 1) * W], mult)

        def phase_c(tb, ob0=0, ob1=None, u0=0, u1=TB):
            """o_proj partials for t block tb over obp range [ob0, ob1) and
            query columns [u0, u1)."""
            if ob1 is None:
                ob1 = NOB // 2
            attnT = attnTs[tb]
            for obp in range(ob0, ob1):
                o_st = outp.tile(
                    [128, 2, TB], bf, tag="o_st", bufs=3,
                    name=f"ost_{tb}_{obp}_{u0}"
                )
                for oi in range(2):
                    ob = 2 * obp + oi
                    ps_o = psum.tile(
                        [128, TB], f32, name=f"pso_{tb}_{ob}_{u0}", tag="bank"
                    )
                    for jc in range(QH):
                        nc.tensor.matmul(
                            ps_o[:, u0:u1],
                            lhsT=wo_sb[:, jc, ob * 128 : (ob + 1) * 128],
                            rhs=attnT[:, jc, u0:u1],
                            start=(jc == 0),
                            stop=(jc == QH - 1),
                        )
                    if oi == 0:
                        nc.vector.tensor_copy(o_st[:, oi, u0:u1], ps_o[:, u0:u1])
                    else:
                        nc.scalar.copy(o_st[:, oi, u0:u1], ps_o[:, u0:u1])
                dst = outr[tb * (NOB // 2) + obp].rearrange(
                    "p (oi u) -> p oi u", u=TB
                )
                if obp % 2 == 0:
                    nc.sync.dma_start(dst[:, :, u0:u1], o_st[:, :, u0:u1])
                else:
                    nc.scalar.dma_start(dst[:, :, u0:u1], o_st[:, :, u0:u1])

        # Software pipeline. PE stream per steady iteration tb:
        #   [o_proj(tb-1)] [attention(tb)] [transpose+rms(tb+1)] [qkv(tb+2)]
        # so every PE phase's dependencies were produced >=1 full PE phase
        # earlier, and the DVE/ACT/DMA chains (rope, softmax norm) hide under
        # dense matmul windows. Last iteration interleaves o_proj(NT-2) around
        # attention(NT-1) to cover the rope/normalize chains at the pipeline
        # drain.
        qkv_mm(0, wide=True)
        tail(0)
        qkv_mm(1)
        nc.scalar.dma_start(wo_sb, wor)
        for tb in range(NT - 1):
            if tb >= 1:
                phase_c(tb - 1, 0, NOB // 4)
            phase_b(tb)
            if tb >= 1:
                phase_c(tb - 1, NOB // 4, NOB // 2)
            tail(tb + 1)
            if tb + 2 < NT:
                qkv_mm(tb + 2)
        # last t block: column-split attention/o_proj so the o_proj matmuls
        # interleave with the second attention half at the pipeline drain
        lb = NT - 1
        HB = TB // 2
        phase_c(lb - 1, 0, NOB // 4)
        phase_b(lb, 0, HB)
        phase_c(lb - 1, NOB // 4, NOB // 2)
        phase_b(lb, HB, TB)
        phase_c(lb, 0, NOB // 2, 0, HB)
        phase_c(lb, 0, NOB // 2, HB, TB)

    nc.compile()
    return nc


def _get_program(T):
    if T not in _PROG_CACHE:
        _PROG_CACHE[T] = _build_program(T)
    return _PROG_CACHE[T]


def _host_prep(positions, hidden_states, wqkv, wo, q_norm_w, k_norm_w):
    """Build the 8 per-core input maps (host-side sharding + table prep)."""
    T = hidden_states.shape[0]
    pos = np.asarray(positions).astype(np.float64)
    hs = np.asarray(hidden_states, dtype=np.float32)
    wqkv = np.asarray(wqkv, dtype=np.float32)
    wo = np.asarray(wo, dtype=np.float32)
    qw = np.asarray(q_norm_w, dtype=np.float64)
    kw = np.asarray(k_norm_w, dtype=np.float64)

    half = D // 2
    inv_freq = 1.0 / (THETA ** (np.arange(0, D, 2, dtype=np.float64) / D))  # [64]
    th = pos[:, None] * inv_freq[None, :]          # [T, 64]
    cos = np.cos(th).T                             # [64, T] float64
    sin = np.sin(th).T

    def tables(w, scale):
        cw = np.empty((D, T), np.float64)
        sw = np.empty((D, T), np.float64)
        cw[:half] = cos * (w[:half, None] * scale)
        cw[half:] = cos * (w[half:, None] * scale)
        # out[d<64] = qn[d]*w[d]*cos - qn[d+64]*w[d+64]*sin  (rot reads qn[d+64])
        sw[:half] = -sin * (w[half:, None] * scale)
        # out[d>=64] = qn[d]*w[d]*cos + qn[d-64]*w[d-64]*sin
        sw[half:] = sin * (w[:half, None] * scale)
        return cw.astype(BF16), sw.astype(BF16)

    cwq, swq = tables(qw, SCALE)
    cwk, swk = tables(kw, 1.0)

    si = np.arange(128)[:, None]
    ui = np.arange(128)[None, :]
    maskd = np.where(ui >= si, 0.0, NEG).astype(np.float32)
    maskw = np.where(ui < si, 0.0, NEG).astype(np.float32)

    # tiled layout: block (tb, cq) = [128, 4*TB]; row p holds c-chunks
    # 4cq..4cq+3 back to back (contiguous per partition)
    NTb, NCq = T // TB, HID // 512
    xT = np.ascontiguousarray(
        hs.T.reshape(NCq, 4, 128, NTb, TB)
        .transpose(3, 0, 2, 1, 4)
        .reshape(NTb * NCq * 128, 4 * TB)
).astype(BF16)

    in_maps = []
    for m in range(M):
        wq_m = wqkv[m * QH * D : (m + 1) * QH * D]            # [512, HID]
        wk_m = wqkv[H * D + m * D : H * D + (m + 1) * D]      # [128, HID]
        wv_m = wqkv[(H + HK) * D + m * D : (H + HK) * D + (m + 1) * D]
        wqkvT_m = np.ascontiguousarray(
            np.concatenate([wq_m, wk_m, wv_m], axis=0).T
        )                                                     # [HID, 768]
        # chunk-tiled like xT: chunk k = [128, 4*768] contiguous per row
        wqkvT_m = np.ascontiguousarray(
            wqkvT_m.reshape(NCq, 4, 128, NJ * 128)
            .transpose(0, 2, 1, 3)
            .reshape(NCq * 128, 4 * NJ * 128)
        ).astype(BF16)
        woT_m = np.ascontiguousarray(
            wo[:, m * QH * D : (m + 1) * QH * D].T
        ).astype(BF16)                                        # [512, HID]
        in_maps.append(
            {
                "xT": xT,
                "wqkvT": wqkvT_m,
                "woT": woT_m,
                "cwq": cwq,
                "swq": swq,
                "cwk": cwk,
                "swk": swk,
                "maskd": maskd,
                "maskw": maskw,
            }
        )
    return in_maps


def _run(in_maps, T, trace=False):
    from concourse import bass_utils

    nc = _get_program(T)
    res = bass_utils.run_bass_kernel_spmd(
        nc, in_maps, core_ids=list(range(M)), trace=trace
    )
    return res


def kernel(positions, hidden_states, wqkv, wo, q_norm_w, k_norm_w, _trace=False):
    T = hidden_states.shape[0]
    in_maps = _host_prep(positions, hidden_states, wqkv, wo, q_norm_w, k_norm_w)
    res = _run(in_maps, T, trace=_trace)
    NTb, NOBp = T // TB, HID // 256
    acc = np.zeros((NTb, NOBp, 128, 2, TB), np.float64)
    for r in res.results:
        acc += r["outT"].astype(np.float64).reshape(NTb, NOBp, 128, 2, TB)
    # untile: out[t, o] with o = (2*obp + oi)*128 + p, t = tb*TB + u
    out = np.ascontiguousarray(
        acc.transpose(0, 4, 1, 3, 2).reshape(T, HID)
    ).astype(np.float32)
    kernel._last_results = res
    return out


# revision 42
# speedup vs baseline: 1.0202x; 1.0202x over previous
"""Trainium2 Bass kernel for Exaone4-style GQA attention block (T=2048, HID=4096,
H=32 q-heads, HK=8 kv-heads, D=128, sliding window 1023, QK-RMSNorm + NeoX RoPE).

Sharding: tensor-parallel over heads across 8 NeuronCores. Core m owns q-heads
[4m, 4m+4) and kv-head m (GQA group-aligned), plus the matching o_proj column
slice; per-core partial outputs are summed on the host (the all-reduce).

Device layout notes:
 - qkv projection is computed transposed ([feature, t]) so attention works in
   the S^T = K^T.T @ Q^T layout; softmax sums over the partition axis are done
   with ones-vector matmuls on the PE, and PV consumes exp(S^T) directly.
 - RMSNorm scale and RoPE are fused via host-precomputed [128, T] cos/sin
   tables (norm weights + 1/sqrt(D) folded in); the RoPE half rotation runs on
   the raw projection (rotation commutes with the per-column norm scale), so
   the SBUF->SBUF rotate DMAs issue as soon as qkv PSUM results are copied out.
 - 1/sqrt(ms+eps) is computed as exp(-0.5*ln(ms+eps)) so every ACT-engine op
   lives in the natural_log_exp table set (no ACT table reloads vs softmax Exp).
 - Schedule keeps the PE dense: per iteration the PE stream is
   [o_proj(tb-1)] [attention(tb)] [v-transpose+rms(tb+1)] [qkv(tb+2)], with
   weight/x DMAs chunk-interleaved so tb=0 compute starts ~4us in.
 - All large matmuls use bf16 operands with fp32 PSUM accumulation.
"""

import sys

import numpy as np

if "/opt/trn_rl_repo" not in sys.path:
    sys.path.insert(0, "/opt/trn_rl_repo")

import ml_dtypes

BF16 = ml_dtypes.bfloat16
# fp8 was evaluated and rejected: the correctness gate is max-abs error and
# the largest outputs (peaked-softmax rows where attn == v) inherit fp8's ~4%
# relative error directly (measured rel 0.048 vs the 0.02 gate). bf16 only.

HID = 4096
H = 32
HK = 8
D = 128
WIN = 1023
THETA = 1000000.0
EPS = 1e-6
SCALE = D ** -0.5
M = 8            # cores
QH = H // M      # q heads per core (4)
NJ = QH + 2      # j-blocks in qkv^T output (4 q + 1 k + 1 v)
TB = 512         # t free-dim block
NEG = -1.0e30

_PROG_CACHE = {}


def _build_program(T):
    """Build the (single-core SPMD) Bass program for sequence length T."""
    from contextlib import ExitStack

    import concourse.bass as bass  # noqa: F401
    import concourse.tile as tile
    from concourse import bacc, mybir

    f32 = mybir.dt.float32
    bf = mybir.dt.bfloat16

    NT = T // TB          # number of t blocks
    NC = HID // 128       # contraction chunks
    NOB = HID // 128      # output row blocks

    nc = bacc.Bacc(
        "TRN2",
        target_bir_lowering=False,
        debug=False,
        enable_asserts=False,
        num_devices=M,
    )

    # x pre-tiled on host: block (tb, cq) = [128, 4*TB], 4 c-chunks interleaved
    # per partition row (contiguous per partition per DMA)
    xT_h = nc.dram_tensor(
        "xT", [(T // TB) * (HID // 512) * 128, 4 * TB], bf, kind="ExternalInput"
    )
    # qkv weights pre-tiled the same way: chunk k = [128, 4*NJ*128] contiguous
    wq_h = nc.dram_tensor(
        "wqkvT", [(NC // 4) * 128, 4 * NJ * 128], bf, kind="ExternalInput"
    )
    wo_h = nc.dram_tensor("woT", [QH * 128, HID], bf, kind="ExternalInput")
    cwq_h = nc.dram_tensor("cwq", [128, T], bf, kind="ExternalInput")
    swq_h = nc.dram_tensor("swq", [128, T], bf, kind="ExternalInput")
    cwk_h = nc.dram_tensor("cwk", [128, T], bf, kind="ExternalInput")
    swk_h = nc.dram_tensor("swk", [128, T], bf, kind="ExternalInput")
    maskd_h = nc.dram_tensor("maskd", [128, 128], f32, kind="ExternalInput")
    maskw_h = nc.dram_tensor("maskw", [128, 128], f32, kind="ExternalInput")
    # out pre-tiled: block (tb, obp) = [128, 2*TB] (ob pairs interleaved per row)
    outT_h = nc.dram_tensor(
        "outT", [(T // TB) * (HID // 256) * 128, 2 * TB], bf, kind="ExternalOutput"
    )

    xTr = xT_h.ap().rearrange("(b p) u -> b p u", p=128)
    wqr = wq_h.ap().rearrange("(k p) u -> k p u", p=128)
    wor = wo_h.ap().rearrange("(jc p) o -> p jc o", p=128)
    outr = outT_h.ap().rearrange("(b p) u -> b p u", p=128)

    mult = mybir.AluOpType.mult
    add = mybir.AluOpType.add
    Exp = mybir.ActivationFunctionType.Exp
    Ln = mybir.ActivationFunctionType.Ln
    Square = mybir.ActivationFunctionType.Square

    with tile.TileContext(nc) as tc, ExitStack() as ctx:
        singles = ctx.enter_context(tc.tile_pool(name="singles", bufs=1))
        persist = ctx.enter_context(tc.tile_pool(name="persist", bufs=1))
        xpool = ctx.enter_context(tc.tile_pool(name="xpool", bufs=3))
        stpool = ctx.enter_context(tc.tile_pool(name="stpool", bufs=1))
        ropep = ctx.enter_context(tc.tile_pool(name="ropep", bufs=1))
        espool = ctx.enter_context(tc.tile_pool(name="espool", bufs=5))
        outp = ctx.enter_context(tc.tile_pool(name="outp", bufs=2))
        smallp = ctx.enter_context(tc.tile_pool(name="smallp", bufs=2))
        # PSUM: every tile is <= one bank; a single tag with 8 rotating slots
        # covers all 8 banks and lets phases overlap freely.
        psum = ctx.enter_context(tc.tile_pool(name="psum", bufs=8, space="PSUM"))
        drp = ctx.enter_context(tc.tile_pool(name="drp", bufs=2, space="DRAM"))

        # ---- tiny resident constants --------------------------------------
        maskd_sb = singles.tile([128, 128], f32)
        maskw_sb = singles.tile([128, 128], f32)
        ones_bf = singles.tile([128, 1], bf)
        nc.vector.memset(ones_bf, 1.0)
        eps_sb = singles.tile([128, 1], f32)
        nc.vector.memset(eps_sb, EPS)

        # rope tables + masks: tiles here, DMAs emitted after qkv_mm(0) so
        # startup bandwidth goes to the first weight/x chunks (tables are
        # first needed by tail(0)'s rope, ~45us in)
        cwq_sb = singles.tile([128, T], bf)
        swq_sb = singles.tile([128, T], bf)
        cwk_sb = singles.tile([128, T], bf)
        swk_sb = singles.tile([128, T], bf)

        # ---- o_proj weights (DMA deferred until after phase_b(0) so the
        # startup bandwidth goes to qkv weights + x) -------------------------
        wo_sb = singles.tile([128, QH, HID], bf)

        # ---- qkv weights: separate chunk tiles so the first matmuls only
        # wait on the first chunk DMA, not the whole load --------------------
        w_chunks = [
            singles.tile([128, 4, NJ * 128], bf, name=f"w_chunk{k}")
            for k in range(NC // 4)
        ]
        w_loaded = [False] * (NC // 4)

        # ---- persistent activations ---------------------------------------
        qT = persist.tile([128, QH, T], bf)     # rope'd+normed q^T
        kT = persist.tile([128, T], bf)         # rope'd+normed k^T
        Vt = persist.tile([128, T // 128, 128], bf)  # v in [s, d] layout

        stages = {}

        def qkv_mm(tb, wide=False):
            """qkv matmuls for t block tb; also emits the psum->stage copies,
            the rope half-rotation DMAs, and the squared-stage muls.

            wide=True runs all 6 j-blocks in one pass over x (6 PSUM banks,
            half the x DMA) - used for tb=0 where nothing else needs PSUM."""
            t0 = tb * TB
            stage = stpool.tile([128, NJ, TB], bf, tag="stage", name=f"stage_{tb}")
            rot = ropep.tile([128, QH + 1, TB], bf, tag="rot", name=f"rot_{tb}")
            sq = stpool.tile([128, QH + 1, TB], bf, tag="sq", name=f"sq_{tb}")
            stages[tb] = (stage, rot, sq)

            groups = [tuple(range(NJ))] if wide else [(0, 1, 2), (3, 4, 5)]
            for js in groups:
                ps_g = [
                    psum.tile([128, TB], f32, name=f"psqkv_{tb}_{j}", tag="bank")
                    for j in js
                ]
                for cq in range(NC // 4):
                    if not w_loaded[cq]:
                        nc.sync.dma_start(
                            w_chunks[cq],
                            wqr[cq].rearrange("p (ci u) -> p ci u", u=NJ * 128),
                        )
                        w_loaded[cq] = True
                    xc = xpool.tile(
                        [128, 4, TB], bf, tag="xc", name=f"xc_{tb}_{js[0]}_{cq}"
                    )
                    nc.sync.dma_start(
                        xc,
                        xTr[tb * (NC // 4) + cq].rearrange("p (ci u) -> p ci u", u=TB),
                    )
                    for ci in range(4):
                        c = 4 * cq + ci
                        for ji, j in enumerate(js):
                            nc.tensor.matmul(
                                ps_g[ji],
                                lhsT=w_chunks[cq][:, ci, j * 128 : (j + 1) * 128],
                                rhs=xc[:, ci, :],
                                start=(c == 0),
                                stop=(c == NC - 1),
                            )
                for ji, j in enumerate(js):
                    if j <= QH:
                        # sq straight from PSUM (ACT Square; DVE cannot read
                        # two PSUM operands) so the rms rowsum does not wait
                        # on the stage copy
                        nc.scalar.activation(sq[:, j], ps_g[ji], Square)
                    # stage copy on ACT (Copy shares the Exp table set) keeps
                    # DVE free for masks/rope
                    nc.scalar.copy(stage[:, j], ps_g[ji])
                    if j <= QH:
                        # rope rotation on the raw projection (scale commutes)
                        nc.gpsimd.dma_start(rot[0:64, j, :], stage[64:128, j, :])
                        nc.gpsimd.dma_start(rot[64:128, j, :], stage[0:64, j, :])
                    else:
                        # v: [d, t] -> [s, d] via DMA transpose (sync queue)
                        for u in range(TB // 128):
                            nc.sync.dma_start(
                                Vt[:, tb * (TB // 128) + u, :],
                                stage[:, j, u * 128 : (u + 1) * 128],
                                transpose=True,
                            )

        def tail(tb):
            """rms scale (PE rowsum + ACT ln/exp + bcast) and rope combine
            for t block tb. PE part is tiny (5 ones-matmuls)."""
            t0 = tb * TB
            ts_ = slice(t0, t0 + TB)
            stage, rot, sq = stages.pop(tb)

            scl = smallp.tile(
                [1, (QH + 1) * TB], bf, tag="scl", bufs=1, name=f"scl_{tb}"
            )
            # batch by function so the ACT table loads once per function group
            # (the table-load pass reloads per canonical set on each switch)
            ps_sss = []
            for j in range(QH + 1):
                ps_ss = psum.tile([1, TB], f32, name=f"psss_{tb}_{j}", tag="bank")
                nc.tensor.matmul(ps_ss, lhsT=ones_bf, rhs=sq[:, j], start=True, stop=True)
                ps_sss.append(ps_ss)
            for j in range(QH + 1):
                # 1/sqrt(x) = exp(-0.5*ln(x)); Ln runs in place in PSUM
                nc.scalar.activation(
                    ps_sss[j], ps_sss[j], Ln, bias=eps_sb[0:1, :], scale=1.0 / D
                )
            for j in range(QH + 1):
                nc.scalar.activation(
                    scl[:, j * TB : (j + 1) * TB], ps_sss[j], Exp, scale=-0.5
                )
            scl_dr = drp.tile([1, (QH + 1) * TB], bf, tag="scl_dr", name=f"scldr_{tb}")
            nc.gpsimd.dma_start(scl_dr, scl)
            sclb = ropep.tile(
                [128, (QH + 1) * TB], bf, tag="sclb", name=f"sclb_{tb}"
            )
            nc.gpsimd.dma_start(sclb, scl_dr.to_broadcast([128, (QH + 1) * TB]))

            # rope combine first (no sclb dependency), scale muls last
            r1s = []
            for j in range(QH + 1):
                cw = cwq_sb if j < QH else cwk_sb
                sw = swq_sb if j < QH else swk_sb
                r1 = ropep.tile([128, TB], f32, tag=f"r1_{j}", name=f"r1_{tb}_{j}")
                nc.vector.tensor_tensor(r1, stage[:, j], cw[:, ts_], mult)
                r2 = ropep.tile([128, TB], f32, tag="r2", bufs=2, name=f"r2_{tb}_{j}")
                nc.vector.tensor_tensor(r2, rot[:, j], sw[:, ts_], mult)
                nc.vector.tensor_tensor(r1, r1, r2, add)
                r1s.append(r1)
            for j in range(QH + 1):
                dest = qT[:, j, ts_] if j < QH else kT[:, ts_]
                nc.vector.tensor_tensor(
                    dest, r1s[j], sclb[:, j * TB : (j + 1) * TB], mult
                )

        attnTs = {}

        def phase_b(tb, u0=0, u1=TB):
            """attention for t block tb, query columns [u0, u1) (attnT kept
            for phase_c).

            Head pairs; the S matmul for block o+1 is emitted before the
            PV/rowsum matmuls for block o so the mask-add->exp round trip
            hides behind PE work. o = sb - 4*tb; o=0 (full col range) FIRST
            so start=True PV/rowsum matmuls cover the whole written range."""
            t0 = tb * TB
            W = u1 - u0
            o_min = max(-8, -((1023 - u0 + 127) // 128))
            o_max = (u1 - 1) // 128
            obs = [0] + [
                o for o in range(o_min, o_max + 1) if o != 0 and 4 * tb + o >= 0
            ]
            nob = len(obs)
            if u0 == 0:
                attnT = outp.tile(
                    [128, QH, TB], bf, tag="attnT", name=f"attnT_{tb}"
                )
                attnTs[tb] = attnT
            else:
                attnT = attnTs[tb]
            rs_dr = drp.tile([1, QH * W], bf, tag="rs_dr", name=f"rsdr_{tb}_{u0}")

            def col_range(o):
                if o >= 0:
                    b0, b1 = 128 * o, TB
                elif o >= -4:
                    b0, b1 = 0, TB
                else:
                    b0, b1 = 0, 128 * (o + 9)
                return max(b0, u0), min(b1, u1)

            for hp in range(QH // 2):
                heads = (2 * hp, 2 * hp + 1)
                pvs = {
                    h: psum.tile([128, TB], f32, name=f"pspv_{tb}_{h}", tag="bank")
                    for h in heads
                }
                rss = {
                    h: psum.tile([1, TB], f32, name=f"psr_{tb}_{h}", tag="bank")
                    for h in heads
                }
                es_tiles = {}

                def emit_S(idx):
                    o = obs[idx]
                    sb = 4 * tb + o
                    c0, c1 = col_range(o)
                    for h in heads:
                        ps_s = psum.tile(
                            [128, TB], f32, name=f"pss_{tb}_{h}_{idx}", tag="bank"
                        )
                        nc.tensor.matmul(
                            ps_s[:, c0:c1],
                            lhsT=kT[:, sb * 128 : (sb + 1) * 128],
                            rhs=qT[:, h, t0 + c0 : t0 + c1],
                            start=True,
                            stop=True,
                        )
                        if o >= 0 and u0 <= 128 * o < u1:
                            # causal strip at cols [128o, 128o+128)
                            us = 128 * o
                            nc.vector.tensor_tensor(
                                ps_s[:, us : us + 128], ps_s[:, us : us + 128],
                                maskd_sb, add,
                            )
                        elif o <= -5 and u0 <= 128 * (o + 8) < u1:
                            # window strip
                            us = 128 * (o + 8)
                            nc.vector.tensor_tensor(
                                ps_s[:, us : us + 128], ps_s[:, us : us + 128],
                                maskw_sb, add,
                            )
                        es = espool.tile(
                            [128, TB], bf, tag="es", name=f"es_{tb}_{h}_{idx}"
                        )
                        nc.scalar.activation(es[:, c0:c1], ps_s[:, c0:c1], Exp)
                        es_tiles[(idx, h)] = es

                emit_S(0)
                if nob > 1:
                    emit_S(1)
                for idx, o in enumerate(obs):
                    if idx + 2 < nob:
                        emit_S(idx + 2)
                    sb = 4 * tb + o
                    c0, c1 = col_range(o)
                    first = idx == 0
                    last = idx == nob - 1
                    for h in heads:
                        es = es_tiles.pop((idx, h))
                        nc.tensor.matmul(
                            pvs[h][:, c0:c1],
                            lhsT=Vt[:, sb, :],
                            rhs=es[:, c0:c1],
                            start=first,
                            stop=last,
                            skip_group_check=True,
                        )
                        nc.tensor.matmul(
                            rss[h][:, c0:c1],
                            lhsT=ones_bf,
                            rhs=es[:, c0:c1],
                            start=first,
                            stop=last,
                            skip_group_check=True,
                        )

                for hi, h in enumerate(heads):
                    # park pre-normalized PV in attnT (normalized in place at
                    # phase end once 1/rowsum is broadcast)
                    nc.vector.tensor_copy(attnT[:, h, u0:u1], pvs[h][:, u0:u1])
                    rtmp = smallp.tile([1, TB], f32, tag="rtmp", bufs=2,
                                       name=f"rtmp_{tb}_{h}_{u0}")
                    nc.vector.reciprocal_approx_fast(rtmp[:, :W], rss[h][:, u0:u1])
                    rtmpb = smallp.tile([1, TB], bf, tag="rtmpb", bufs=2,
                                        name=f"rtmpb_{tb}_{h}_{u0}")
                    nc.vector.tensor_copy(rtmpb[:, :W], rtmp[:, :W])
                    nc.gpsimd.dma_start(rs_dr[:, h * W : (h + 1) * W], rtmpb[:, :W])

            rb4 = ropep.tile([128, QH * TB], bf, tag="rb4", name=f"rb4_{tb}_{u0}")
            nc.gpsimd.dma_start(rb4[:, : QH * W], rs_dr.to_broadcast([128, QH * W]))
            for h in range(QH):
                nc.vector.tensor_tensor(attnT[:, h, u0:u1], attnT[:, h, u0:u1],
                                        rb4[:, h * W : (h + 1) * W], mult)

        def phase_c(tb, ob0=0, ob1=None, u0=0, u1=TB):
            """o_proj partials for t block tb over obp range [ob0, ob1) and
            query columns [u0, u1)."""
            if ob1 is None:
                ob1 = NOB // 2
            attnT = attnTs[tb]
            for obp in range(ob0, ob1):
                o_st = outp.tile(
                    [128, 2, TB], bf, tag="o_st", bufs=3,
                    name=f"ost_{tb}_{obp}_{u0}"
                )
                for oi in range(2):
                    ob = 2 * obp + oi
                    ps_o = psum.tile(
                        [128, TB], f32, name=f"pso_{tb}_{ob}_{u0}", tag="bank"
                    )
                    for jc in range(QH):
                        nc.tensor.matmul(
                            ps_o[:, u0:u1],
                            lhsT=wo_sb[:, jc, ob * 128 : (ob + 1) * 128],
                            rhs=attnT[:, jc, u0:u1],
                            start=(jc == 0),
                            stop=(jc == QH - 1),
                        )
                    if oi == 0:
                        nc.vector.tensor_copy(o_st[:, oi, u0:u1], ps_o[:, u0:u1])
                    elif ob0 == 0:
                        # first half-phase: ACT copy (DVE must stay clear for
                        # the imminent softmax mask adds)
                        nc.scalar.copy(o_st[:, oi, u0:u1], ps_o[:, u0:u1])
                    else:
                        # second half-phase: DVE copy (eases the ACT queue
                        # ahead of the next tail/qkv stage copies)
                        nc.vector.tensor_copy(o_st[:, oi, u0:u1], ps_o[:, u0:u1])
                dst = outr[tb * (NOB // 2) + obp].rearrange(
                    "p (oi u) -> p oi u", u=TB
                )
                if obp % 2 == 0:
                    nc.sync.dma_start(dst[:, :, u0:u1], o_st[:, :, u0:u1])
                else:
                    nc.scalar.dma_start(dst[:, :, u0:u1], o_st[:, :, u0:u1])

        # Software pipeline. PE stream per steady iteration tb:
        #   [o_proj(tb-1)] [attention(tb)] [transpose+rms(tb+1)] [qkv(tb+2)]
        # so every PE phase's dependencies were produced >=1 full PE phase
        # earlier, and the DVE/ACT/DMA chains (rope, softmax norm) hide under
        # dense matmul windows. Last iteration interleaves o_proj(NT-2) around
        # attention(NT-1) to cover the rope/normalize chains at the pipeline
        # drain.
        qkv_mm(0, wide=True)
        # deferred constant loads: land during qkv(0)'s matmul stream
        nc.gpsimd.dma_start(cwq_sb, cwq_h.ap())
        nc.gpsimd.dma_start(swq_sb, swq_h.ap())
        nc.gpsimd.dma_start(cwk_sb, cwk_h.ap())
        nc.gpsimd.dma_start(swk_sb, swk_h.ap())
        nc.scalar.dma_start(maskd_sb, maskd_h.ap())
        nc.scalar.dma_start(maskw_sb, maskw_h.ap())
        tail(0)
        qkv_mm(1)
        nc.scalar.dma_start(wo_sb, wor)
        for tb in range(NT):
            if tb >= 1:
                phase_c(tb - 1, 0, NOB // 4)
            phase_b(tb)
            if tb >= 1:
                phase_c(tb - 1, NOB // 4, NOB // 2)
            if tb + 1 < NT:
                tail(tb + 1)
            if tb + 2 < NT:
                qkv_mm(tb + 2)
        phase_c(NT - 1)

    nc.compile()
    return nc


def _get_program(T):
    if T not in _PROG_CACHE:
        _PROG_CACHE[T] = _build_program(T)
    return _PROG_CACHE[T]


def _host_prep(positions, hidden_states, wqkv, wo, q_norm_w, k_norm_w):
    """Build the 8 per-core input maps (host-side sharding + table prep)."""
    T = hidden_states.shape[0]
    pos = np.asarray(positions).astype(np.float64)
    hs = np.asarray(hidden_states, dtype=np.float32)
    wqkv = np.asarray(wqkv, dtype=np.float32)
    wo = np.asarray(wo, dtype=np.float32)
    qw = np.asarray(q_norm_w, dtype=np.float64)
    kw = np.asarray(k_norm_w, dtype=np.float64)

    half = D // 2
    inv_freq = 1.0 / (THETA ** (np.arange(0, D, 2, dtype=np.float64) / D))  # [64]
    th = pos[:, None] * inv_freq[None, :]          # [T, 64]
    cos = np.cos(th).T                             # [64, T] float64
    sin = np.sin(th).T

    def tables(w, scale):
        cw = np.empty((D, T), np.float64)
        sw = np.empty((D, T), np.float64)
        cw[:half] = cos * (w[:half, None] * scale)
        cw[half:] = cos * (w[half:, None] * scale)
        # out[d<64] = qn[d]*w[d]*cos - qn[d+64]*w[d+64]*sin  (rot reads qn[d+64])
        sw[:half] = -sin * (w[half:, None] * scale)
        # out[d>=64] = qn[d]*w[d]*cos + qn[d-64]*w[d-64]*sin
        sw[half:] = sin * (w[:half, None] * scale)
        return cw.astype(BF16), sw.astype(BF16)

    cwq, swq = tables(qw, SCALE)
    cwk, swk = tables(kw, 1.0)

    si = np.arange(128)[:, None]
    ui = np.arange(128)[None, :]
    maskd = np.where(ui >= si, 0.0, NEG).astype(np.float32)
    maskw = np.where(ui < si, 0.0, NEG).astype(np.float32)

    # tiled layout: block (tb, cq) = [128, 4*TB]; row p holds c-chunks
    # 4cq..4cq+3 back to back (contiguous per partition)
    NTb, NCq = T // TB, HID // 512
    xT = np.ascontiguousarray(
        hs.T.reshape(NCq, 4, 128, NTb, TB)
        .transpose(3, 0, 2, 1, 4)
        .reshape(NTb * NCq * 128, 4 * TB)
).astype(BF16)

    in_maps = []
    for m in range(M):
        wq_m = wqkv[m * QH * D : (m + 1) * QH * D]            # [512, HID]
        wk_m = wqkv[H * D + m * D : H * D + (m + 1) * D]      # [128, HID]
        wv_m = wqkv[(H + HK) * D + m * D : (H + HK) * D + (m + 1) * D]
        wqkvT_m = np.ascontiguousarray(
            np.concatenate([wq_m, wk_m, wv_m], axis=0).T
        )                                                     # [HID, 768]
        # chunk-tiled like xT: chunk k = [128, 4*768] contiguous per row
        wqkvT_m = np.ascontiguousarray(
            wqkvT_m.reshape(NCq, 4, 128, NJ * 128)
            .transpose(0, 2, 1, 3)
            .reshape(NCq * 128, 4 * NJ * 128)
        ).astype(BF16)
        woT_m = np.ascontiguousarray(
            wo[:, m * QH * D : (m + 1) * QH * D].T
        ).astype(BF16)                                        # [512, HID]
        in_maps.append(
            {
                "xT": xT,
                "wqkvT": wqkvT_m,
                "woT": woT_m,
                "cwq": cwq,
                "swq": swq,
                "cwk": cwk,
                "swk": swk,
                "maskd": maskd,
                "maskw": maskw,
            }
        )
    return in_maps


def _run(in_maps, T, trace=False):
    from concourse import bass_utils

    nc = _get_program(T)
    res = bass_utils.run_bass_kernel_spmd(
        nc, in_maps, core_ids=list(range(M)), trace=trace
    )
    return res


def kernel(positions, hidden_states, wqkv, wo, q_norm_w, k_norm_w, _trace=False):
    T = hidden_states.shape[0]
    in_maps = _host_prep(positions, hidden_states, wqkv, wo, q_norm_w, k_norm_w)
    res = _run(in_maps, T, trace=_trace)
    NTb, NOBp = T // TB, HID // 256
    acc = np.zeros((NTb, NOBp, 128, 2, TB), np.float64)
    for r in res.results:
        acc += r["outT"].astype(np.float64).reshape(NTb, NOBp, 128, 2, TB)
    # untile: out[t, o] with o = (2*obp + oi)*128 + p, t = tb*TB + u
    out = np.ascontiguousarray(
        acc.transpose(0, 4, 1, 3, 2).reshape(T, HID)
    ).astype(np.float32)
    kernel._last_results = res
    return out
